# revision 8
# baseline (speedup 1.0000x reference)
"""LoRA-QKV fused projection kernel for 8 trn2 NeuronCores.

Math: out = x @ W.T + b, with LoRA updates folded into W on the host:
  (x @ A_q.T) @ B_q.T == x @ (B_q @ A_q).T   (exact linear-algebra identity)
so W_eff = W + scaling * pad(B_q@A_q, B_v@A_v) and the device runs ONE GEMM.

Sharding: data-parallel over tokens. x is (32,1024,1024) -> 32768 tokens of
dim 1024; each of the 8 cores computes a disjoint 4096-token slice of the
[32768, 3072] output. W_eff/bias replicated. No collectives.

Device kernel (per core): out[4096, 3072] = xT.T @ wT + bias
  - lhsT (stationary) = x^T tiles [128k, 128tok], host pre-transposed/blocked
  - rhs  (moving)     = W_eff^T tiles [128k, 512f], resident in SBUF
  - PSUM accumulates over the 8 k-tiles; DVE fuses bias-add with PSUM->SBUF.
Compute dtype float32r: full PE rate (1 col/cycle at N>=256) vs fp32's 1/4.
"""

import os

import numpy as np

import concourse.bass as bass
import concourse.mybir as mybir
import concourse.tile as tile
from concourse import bacc, bass_utils
from concourse.bass import ts

NCORES = 8
B, N, D = 32, 1024, 1024
TOK = B * N          # 32768 tokens
TPC = TOK // NCORES  # 4096 tokens per core
OUTF = 3 * D         # 3072 output features
SCALING = 1.0        # alpha/rank = 16/16

P = 128
KT = D // P          # 8 k-tiles
NF = 512             # matmul free dim / PSUM bank
NT = OUTF // NF      # 6 n-tiles
MT = TPC // P        # 32 m-tiles

COMPUTE_DT = os.environ.get("K_DTYPE", "f32r")  # f32r | bf16 | fp32
STORE_DT = os.environ.get("K_STORE_DT", "fp32")  # fp32 | bf16
STORE_ENG = os.environ.get("K_STORE_ENG", "sync")  # sync | scalar | alt
TRACE = os.environ.get("K_TRACE", "0") == "1"
# bench-only: repeat the compute loop R times inside the NEFF to amplify
# device time over dispatch noise. Grading path always uses 1.
REPEAT = int(os.environ.get("K_REPEAT", "1"))

_DT_MAP = {
    "f32r": mybir.dt.float32r,
    "bf16": mybir.dt.bfloat16,
    "fp32": mybir.dt.float32,
}

_MODULE_CACHE = {}
LAST_RESULTS = None


def _build_module(
    dt_in,
    repeat=1,
    kt_lim=None,
    store_nt=None,
    x_once=False,
    merge_store=True,
    store_dt="fp32",
    store_eng="sync",
):
    """kt_lim/store_nt/x_once are bench-only ablations (wrong results)."""
    if kt_lim is None:
        kt_lim = KT
    if store_nt is None:
        store_nt = NT
    out_mydt = mybir.dt.bfloat16 if store_dt == "bf16" else mybir.dt.float32
    nc = bacc.Bacc(
        "TRN2",
        target_bir_lowering=False,
        debug=False,
        num_devices=NCORES,
    )
    # blocked x^T: [m-tile, k-partition, k-tile, token] -> contiguous 512KB/tile
    xp = nc.dram_tensor("xp", [MT, P, KT, P], dt_in, kind="ExternalInput").ap()
    # blocked W_eff^T: [k-partition, k-tile, feature]
    wp = nc.dram_tensor("wp", [P, KT, OUTF], dt_in, kind="ExternalInput").ap()
    # bias replicated across partitions
    bias = nc.dram_tensor(
        "bias", [P, OUTF], mybir.dt.float32, kind="ExternalInput"
    ).ap()
    out = nc.dram_tensor(
        "out", [TPC, OUTF], out_mydt, kind="ExternalOutput"
    ).ap()
    out3 = out.rearrange("(mo p) f -> p mo f", p=P)

    with tile.TileContext(nc) as tc:
        with (
            tc.tile_pool(name="w", bufs=1) as wpool,
            tc.tile_pool(name="bias", bufs=1) as bpool,
            tc.tile_pool(name="x", bufs=3) as xpool,
            tc.tile_pool(name="o", bufs=3) as opool,
            tc.tile_pool(name="acc", bufs=1) as accpool,
            tc.tile_pool(name="ps", bufs=8, space="PSUM") as pspool,
        ):
            # per-k W tiles so matmuls can start as soon as each 1.5MB lands
            w_tiles = []
            for k in range(KT):
                wk = wpool.tile([P, OUTF], dt_in, tag=f"w{k}")
                nc.sync.dma_start(wk[:], wp[:, k, :])
                w_tiles.append(wk)
            bt = bpool.tile([P, OUTF], mybir.dt.float32, tag="bias")
            nc.sync.dma_start(bt[:], bias[:])

            acc = None
            if store_nt < NT:
                acc = accpool.tile([P, NF], mybir.dt.float32, tag="acc", name="acc")
                nc.vector.tensor_copy(out=acc[:], in_=bt[:, ts(0, NF)])
            xm0 = None
            for rep in range(repeat):
              for m in range(MT):
                if x_once:
                    if xm0 is None:
                        xm0 = xpool.tile([P, KT, P], dt_in, name="xm0", tag="xm")
                        nc.sync.dma_start(xm0[:], xp[0])
                    xm = xm0
                else:
                    xm = xpool.tile([P, KT, P], dt_in, name=f"xm_{rep}_{m}", tag="xm")
                    nc.sync.dma_start(xm[:], xp[m])
                # k outer / n inner: the stationary lhsT xm[:,k,:] is reused
                # across 6 consecutive matmuls; 6 PSUM banks accumulate in
                # parallel across the k loop.
                pss = [
                    pspool.tile(
                        [P, NF], mybir.dt.float32, tag="ps", name=f"ps_{rep}_{m}_{n}"
                    )
                    for n in range(NT)
                ]
                for k in range(kt_lim):
                    for n in range(NT):
                        nc.tensor.matmul(
                            pss[n][:],
                            xm[:, k, :],
                            w_tiles[k][:, ts(n, NF)],
                            start=(k == 0),
                            stop=(k == kt_lim - 1),
                        )
                if store_nt == NT and merge_store:
                    # one [128, 3072] staging tile per m-tile: the DRAM store
                    # becomes a single fully-contiguous 1.5 MiB transfer
                    om = opool.tile(
                        [P, OUTF], out_mydt, tag="ot", name=f"om_{rep}_{m}"
                    )
                    for n in range(NT):
                        nc.vector.tensor_add(
                            out=om[:, ts(n, NF)], in0=pss[n][:], in1=bt[:, ts(n, NF)]
                        )
                    if store_eng == "scalar" or (store_eng == "alt" and m % 2 == 1):
                        nc.scalar.dma_start(out3[:, m, :], om[:])
                    else:
                        nc.sync.dma_start(out3[:, m, :], om[:])
                else:
                    for n in range(NT):
                        if n < store_nt:
                            ot = opool.tile(
                                [P, NF],
                                mybir.dt.float32,
                                tag="ot",
                                name=f"ot_{rep}_{m}_{n}",
                            )
                            nc.vector.tensor_add(
                                out=ot[:], in0=pss[n][:], in1=bt[:, ts(n, NF)]
                            )
                            nc.sync.dma_start(out3[:, m, ts(n, NF)], ot[:])
                        else:
                            # consume psum without a DRAM store (keeps DCE away)
                            nc.vector.tensor_add(
                                out=acc[:], in0=acc[:], in1=pss[n][:]
                            )
            if acc is not None:
                nc.sync.dma_start(out3[:, 0, ts(0, NF)], acc[:])
    nc.compile()
    return nc


def _get_module(dtype_key, repeat=None):
    if repeat is None:
        repeat = REPEAT
    key = (dtype_key, repeat, STORE_DT, STORE_ENG)
    if key not in _MODULE_CACHE:
        _MODULE_CACHE[key] = _build_module(
            _DT_MAP[dtype_key], repeat, store_dt=STORE_DT, store_eng=STORE_ENG
        )
    return _MODULE_CACHE[key]


def prepare_in_maps(x, W, b, A_q, B_q, A_v, B_v):
    x = np.asarray(x)
    W = np.asarray(W)
    b = np.asarray(b)

    # Fold LoRA into W (in fp64 to keep the fold exact at fp32 resolution)
    W_eff = W.astype(np.float64).copy()
    W_eff[:D] += SCALING * (
        np.asarray(B_q).astype(np.float64) @ np.asarray(A_q).astype(np.float64)
    )
    W_eff[2 * D:] += SCALING * (
        np.asarray(B_v).astype(np.float64) @ np.asarray(A_v).astype(np.float64)
    )
    W_eff = W_eff.astype(np.float32)

    np_dt = np.float32
    if COMPUTE_DT == "bf16":
        import ml_dtypes

        np_dt = ml_dtypes.bfloat16

    # blocked W_eff^T: wp[ki, ko, f] = W_eff[f, ko*128+ki]
    wp = np.ascontiguousarray(
        W_eff.T.reshape(KT, P, OUTF).transpose(1, 0, 2)
    ).astype(np_dt)
    bias_rep = np.ascontiguousarray(
        np.broadcast_to(b.astype(np.float32), (P, OUTF))
    )

    x_flat = x.reshape(TOK, D)
    in_maps = []
    for c in range(NCORES):
        xc = x_flat[c * TPC : (c + 1) * TPC]
        # xp[m, ki, ko, t] = xc[m*128+t, ko*128+ki]
        xpn = np.ascontiguousarray(
            xc.reshape(MT, P, KT, P).transpose(0, 3, 2, 1)
        ).astype(np_dt)
        in_maps.append({"xp": xpn, "wp": wp, "bias": bias_rep})
    return in_maps


def kernel(x, W, b, A_q, B_q, A_v, B_v):
    global LAST_RESULTS
    in_maps = prepare_in_maps(x, W, b, A_q, B_q, A_v, B_v)

    nc = _get_module(COMPUTE_DT)
    res = bass_utils.run_bass_kernel_spmd(
        nc, in_maps, core_ids=list(range(NCORES)), trace=TRACE
    )
    LAST_RESULTS = res

    out = np.concatenate([r["out"] for r in res.results], axis=0)
    if out.dtype != np.float32:
        out = out.astype(np.float32)
    return out.reshape(B, N, OUTF)

